# revision 28
# baseline (speedup 1.0000x reference)
"""YOLO-style detector decode kernel for Trainium2 (8 NeuronCores, SPMD).

Sharding: pure data parallel over the batch dim (128 -> 16 per core).

Per-core layout: for each head (13/26/52) the 45 channels are 3 anchors x 15
roles (iou, dx, dy, dw, dh, 10 class scores).  Each role is gathered into an
SBUF tile [P, F] where the partition dim enumerates (g, b, a) (g = half of the
cell space, to use 96 of 128 partitions) and the free dim enumerates cells.
Decode is then elementwise over role planes:

  pre   = grid*stride + stride*dx          (iota grid + ACT copy-scale + DVE add)
  half  = exp(dw + ln(anchor/2))           (single ACT op, bias folds the mul)
  x1/x2 = pre -/+ half
  mask  = iou > thresh
  kind  = tournament argmax over the 10 class planes

Outputs are written as dense planes ob[6, B, 3, HW] (+ mask [B, 3, HW]) per
head; the host interleaves them into the reference [N, 6] ordering (cheap
numpy transpose, keeps every DMA fully contiguous).
"""

import numpy as np

import concourse.bass as bass
import concourse.bacc as bacc
import concourse.mybir as mybir
from concourse.tile import TileContext
from concourse.bass_utils import run_bass_kernel_spmd

F32 = mybir.dt.float32
U8 = mybir.dt.uint8
ALU = mybir.AluOpType
ACTF = mybir.ActivationFunctionType

NCORES = 8
B = 128
BL = B // NCORES  # 16

# name, H(=W), stride, G (cell-space split to widen partitions), free chunk
HEADS = [
    ("13", 13, 32, 1, 169),
    ("26", 26, 16, 2, 338),
    ("52", 52, 8, 2, 676),
]


def _build_nc(reps=1, variant="full"):
    # Bacc (not raw Bass): its compile() legalizes multi-wait sync_info into
    # event semaphores (hardware allows one wait per instruction).
    # reps>1 wraps the body in a For_i loop -- used only for benchmarking
    # (fixed host/proxy overhead cancels between reps=1 and reps=R runs).
    # variant: "full" | "dma" (no compute) | "compute" (no role/out DMAs).
    import contextlib
    nc = bacc.Bacc("TRN2", target_bir_lowering=False)
    ins = {}
    obs = {}
    oms = {}
    for name, H, S, G, FC in HEADS:
        HW = H * H
        ins[name] = nc.declare_dram_parameter(f"in{name}", [BL, 45, HW], F32, isOutput=False)
        obs[name] = nc.declare_dram_parameter(f"ob{name}", [6, BL, 3, HW], F32, isOutput=True)
        oms[name] = nc.declare_dram_parameter(f"om{name}", [BL, 3, HW], U8, isOutput=True)
    consts = nc.declare_dram_parameter("consts", [96, 8], F32, isOutput=False)

    with TileContext(nc) as tc:
        with (
            tc.tile_pool(name="persist", bufs=1) as ppool,
            tc.tile_pool(name="work", bufs=2) as wpool,
            tc.tile_pool(name="scratch", bufs=1) as spool,
        ):
            consts_t = ppool.tile([96, 8], F32, tag="consts")
            nc.sync.dma_start(out=consts_t[:, :], in_=consts[:])

            # Per-head grid tiles (stride-scaled cell coordinates), built once
            # on gpsimd via iota.  Values <= 408, exact in f32.
            grids = {}
            for name, H, S, G, FC in HEADS:
                P = 48 * G
                H2 = H // G
                F = H2 * H
                gx = ppool.tile([P, F], F32, tag=f"gx{name}")
                gy = ppool.tile([P, F], F32, tag=f"gy{name}")
                nc.gpsimd.iota(
                    gx[:, :], pattern=[[0, H2], [S, H]], base=0,
                    channel_multiplier=0, allow_small_or_imprecise_dtypes=True,
                )
                # iota can't start at partition 48 (and partition windows must
                # be 32-aligned), so write the g=0 pattern everywhere and add
                # the g=1 half-offset (H2*S = 208 for both split heads) from
                # consts col 7 as a one-time per-partition scalar.
                nc.gpsimd.iota(
                    gy[:, :], pattern=[[S, H2], [0, H]], base=0,
                    channel_multiplier=0, allow_small_or_imprecise_dtypes=True,
                )
                if G == 2:
                    nc.vector.tensor_scalar(
                        out=gy[:, :], in0=gy[:, :],
                        scalar1=consts_t[0:P, 7:8], scalar2=None, op0=ALU.add,
                    )
                grids[name] = (gx, gy)

            rep_ctx = tc.For_i(0, reps, 1) if reps > 1 else contextlib.nullcontext()
            with rep_ctx:
                _emit_body(nc, tc, wpool, spool, consts_t, grids, ins, obs, oms, variant)
    nc.compile()
    return nc


def _emit_body(nc, tc, wpool, spool, consts_t, grids, ins, obs, oms, variant="full"):
            for hi, (name, H, S, G, FC) in enumerate(HEADS):
                HW = H * H
                P = 48 * G
                F = HW // G
                gx, gy = grids[name]
                in_r = ins[name][:].rearrange("b (a r) (g f) -> g b a r f", a=3, g=G)
                ob_r = obs[name][:].rearrange("r b a (g f) -> r g b a f", g=G)
                om_r = oms[name][:].rearrange("b a (g f) -> g b a f", g=G)
                cw, ch = 2 * hi, 2 * hi + 1

                for cs in range(0, F, FC):
                    ce = cs + FC
                    t = []
                    for k in range(15):
                        tk = wpool.tile([P, FC], F32, tag=f"r{k}")
                        if variant != "compute":
                            nc.sync.dma_start(out=tk[:, :], in_=in_r[:, :, :, k, cs:ce])
                        t.append(tk)

                    if variant == "dma":
                        msk = wpool.tile([P, FC], U8, tag="msk")
                        nc.gpsimd.memset(msk[:, :], 0)
                        for ri in range(6):
                            nc.sync.dma_start(out=ob_r[ri, :, :, :, cs:ce], in_=t[ri][:, :])
                        nc.sync.dma_start(out=om_r[:, :, :, cs:ce], in_=msk[:, :])
                        continue

                    hw2 = wpool.tile([P, FC], F32, tag="hw2")
                    hh2 = wpool.tile([P, FC], F32, tag="hh2")
                    # ACT runs only Exp (single function table, no reloads);
                    # bias folds the anchor/2 multiplier: a/2*e^x = e^(x+ln(a/2))
                    nc.scalar.activation(hw2[:, :], t[3][:, :], ACTF.Exp, bias=consts_t[0:P, cw:cw + 1])
                    nc.scalar.activation(hh2[:, :], t[4][:, :], ACTF.Exp, bias=consts_t[0:P, ch:ch + 1])

                    # pre = grid + S*d, fused: (d * S) + grid, in place on t1/t2
                    nc.vector.scalar_tensor_tensor(
                        t[1][:, :], t[1][:, :], float(S), gx[:, cs:ce], ALU.mult, ALU.add)
                    nc.vector.scalar_tensor_tensor(
                        t[2][:, :], t[2][:, :], float(S), gy[:, cs:ce], ALU.mult, ALU.add)
                    x1 = wpool.tile([P, FC], F32, tag="x1")
                    y1 = wpool.tile([P, FC], F32, tag="y1")
                    nc.vector.tensor_tensor(x1[:, :], t[1][:, :], hw2[:, :], ALU.subtract)
                    nc.vector.tensor_tensor(y1[:, :], t[2][:, :], hh2[:, :], ALU.subtract)
                    # x2/y2 in place on t1/t2
                    nc.vector.tensor_tensor(t[1][:, :], t[1][:, :], hw2[:, :], ALU.add)
                    nc.vector.tensor_tensor(t[2][:, :], t[2][:, :], hh2[:, :], ALU.add)

                    # mask = iou > thresh (gpsimd, off the DVE critical path)
                    msk = wpool.tile([P, FC], U8, tag="msk")
                    nc.gpsimd.tensor_scalar(
                        out=msk[:, :], in0=t[0][:, :],
                        scalar1=consts_t[0:P, 6:7], scalar2=None, op0=ALU.is_gt)

                    # tournament argmax over class planes t[5..14]
                    c = t[5:]
                    m = []   # running max planes (in place on even class planes)
                    ix = []  # running argmax planes
                    for i in range(5):
                        gt = spool.tile([P, FC], U8, tag=f"gt{i}")
                        idx = spool.tile([P, FC], F32, tag=f"idx{i}")
                        nc.vector.tensor_tensor(gt[:, :], c[2 * i + 1][:, :], c[2 * i][:, :], ALU.is_gt)
                        nc.vector.tensor_tensor(c[2 * i][:, :], c[2 * i][:, :], c[2 * i + 1][:, :], ALU.max)
                        nc.vector.tensor_scalar(
                            out=idx[:, :], in0=gt[:, :],
                            scalar1=float(2 * i), scalar2=None, op0=ALU.add,
                        )
                        m.append(c[2 * i])
                        ix.append(idx)

                    def merge(i, j):
                        g2 = spool.tile([P, FC], U8, tag="gm")
                        nc.vector.tensor_tensor(g2[:, :], m[j][:, :], m[i][:, :], ALU.is_gt)
                        nc.vector.copy_predicated(ix[i][:, :], g2[:, :], ix[j][:, :])
                        nc.vector.tensor_tensor(m[i][:, :], m[i][:, :], m[j][:, :], ALU.max)

                    merge(0, 1)
                    merge(2, 3)
                    merge(0, 2)
                    merge(0, 4)

                    if variant != "compute":
                        for ri, plane in ((0, t[0]), (1, x1), (2, y1), (3, t[1]), (4, t[2]), (5, ix[0])):
                            nc.sync.dma_start(out=ob_r[ri, :, :, :, cs:ce], in_=plane[:, :])
                        nc.sync.dma_start(out=om_r[:, :, :, cs:ce], in_=msk[:, :])


_NC_CACHE = {}


def _get_nc(reps=1, variant="full"):
    key = (reps, variant)
    if key not in _NC_CACHE:
        _NC_CACHE[key] = _build_nc(reps, variant)
    return _NC_CACHE[key]


def _host_inputs(output13, output26, output52, anchors13, anchors26, anchors52, thresh):
    consts = np.zeros((96, 8), np.float32)
    amod = np.arange(96) % 3
    for hi, anc in enumerate((anchors13, anchors26, anchors52)):
        anc = np.asarray(anc, np.float32)
        consts[:, 2 * hi] = np.log(anc[:, 0] / 2.0)[amod]
        consts[:, 2 * hi + 1] = np.log(anc[:, 1] / 2.0)[amod]
    consts[:, 6] = np.float32(thresh)
    consts[:, 7] = np.where(np.arange(96) >= 48, 208.0, 0.0)
    full = {"13": np.asarray(output13, np.float32),
            "26": np.asarray(output26, np.float32),
            "52": np.asarray(output52, np.float32)}
    in_maps = []
    for cid in range(NCORES):
        m = {"consts": consts}
        for name in full:
            sl = np.ascontiguousarray(full[name][cid * BL:(cid + 1) * BL])
            m[f"in{name}"] = sl.reshape(BL, 45, -1)
        in_maps.append(m)
    return in_maps


def _assemble(results):
    boxes_parts = []
    mask_parts = []
    for name, H, S, G, FC in HEADS:
        ob = np.concatenate([r[f"ob{name}"] for r in results], axis=1)  # [6,128,3,HW]
        om = np.concatenate([r[f"om{name}"] for r in results], axis=0)  # [128,3,HW]
        boxes_parts.append(np.ascontiguousarray(ob.transpose(1, 3, 2, 0)).reshape(-1, 6))
        mask_parts.append((om.transpose(0, 2, 1) != 0).reshape(-1))
    return np.concatenate(boxes_parts, 0), np.concatenate(mask_parts, 0)


def _run(trace=False, **inputs):
    nc = _get_nc()
    in_maps = _host_inputs(**inputs)
    res = run_bass_kernel_spmd(nc, in_maps, list(range(NCORES)), trace=trace)
    out = _assemble(res.results)
    return out, res


def kernel(**inputs):
    out, _ = _run(trace=False, **inputs)
    return out


def kernel_traced(**inputs):
    return _run(trace=True, **inputs)


# revision 29
# speedup vs baseline: 1.0326x; 1.0326x over previous
"""YOLO-style detector decode kernel for Trainium2 (8 NeuronCores, SPMD).

Sharding: pure data parallel over the batch dim (128 -> 16 per core).

Per-core layout: for each head (13/26/52) the 45 channels are 3 anchors x 15
roles (iou, dx, dy, dw, dh, 10 class scores).  Each role is gathered into an
SBUF tile [P, F] where the partition dim enumerates (g, b, a) (g = half of the
cell space, to use 96 of 128 partitions) and the free dim enumerates cells.
Decode is then elementwise over role planes:

  pre   = grid*stride + stride*dx          (iota grid + ACT copy-scale + DVE add)
  half  = exp(dw + ln(anchor/2))           (single ACT op, bias folds the mul)
  x1/x2 = pre -/+ half
  mask  = iou > thresh
  kind  = tournament argmax over the 10 class planes

Outputs are written as dense planes ob[6, B, 3, HW] (+ mask [B, 3, HW]) per
head; the host interleaves them into the reference [N, 6] ordering (cheap
numpy transpose, keeps every DMA fully contiguous).
"""

import numpy as np

import concourse.bass as bass
import concourse.bacc as bacc
import concourse.mybir as mybir
from concourse.tile import TileContext
from concourse.bass_utils import run_bass_kernel_spmd

F32 = mybir.dt.float32
U8 = mybir.dt.uint8
ALU = mybir.AluOpType
ACTF = mybir.ActivationFunctionType

NCORES = 8
B = 128
BL = B // NCORES  # 16

# name, H(=W), stride, G (cell-space split to widen partitions), free chunk
HEADS = [
    ("13", 13, 32, 1, 169),
    ("26", 26, 16, 2, 338),
    ("52", 52, 8, 2, 676),
]


def _build_nc(reps=1, variant="full"):
    # Bacc (not raw Bass): its compile() legalizes multi-wait sync_info into
    # event semaphores (hardware allows one wait per instruction).
    # reps>1 wraps the body in a For_i loop -- used only for benchmarking
    # (fixed host/proxy overhead cancels between reps=1 and reps=R runs).
    # variant: "full" | "dma" (no compute) | "compute" (no role/out DMAs).
    import contextlib
    nc = bacc.Bacc("TRN2", target_bir_lowering=False)
    ins = {}
    obs = {}
    oms = {}
    for name, H, S, G, FC in HEADS:
        HW = H * H
        ins[name] = nc.declare_dram_parameter(f"in{name}", [BL, 45, HW], F32, isOutput=False)
        obs[name] = nc.declare_dram_parameter(f"ob{name}", [6, BL, 3, HW], F32, isOutput=True)
        oms[name] = nc.declare_dram_parameter(f"om{name}", [BL, 3, HW], U8, isOutput=True)
    consts = nc.declare_dram_parameter("consts", [96, 8], F32, isOutput=False)

    with TileContext(nc) as tc:
        with (
            tc.tile_pool(name="persist", bufs=1) as ppool,
            tc.tile_pool(name="work", bufs=2) as wpool,
            tc.tile_pool(name="scratch", bufs=1) as spool,
        ):
            consts_t = ppool.tile([96, 8], F32, tag="consts")
            nc.sync.dma_start(out=consts_t[:, :], in_=consts[:])

            # Per-head grid tiles (stride-scaled cell coordinates), built once
            # on gpsimd via iota.  Values <= 408, exact in f32.
            grids = {}
            for name, H, S, G, FC in HEADS:
                P = 48 * G
                H2 = H // G
                F = H2 * H
                gx = ppool.tile([P, F], F32, tag=f"gx{name}")
                gy = ppool.tile([P, F], F32, tag=f"gy{name}")
                nc.gpsimd.iota(
                    gx[:, :], pattern=[[0, H2], [S, H]], base=0,
                    channel_multiplier=0, allow_small_or_imprecise_dtypes=True,
                )
                # iota can't start at partition 48 (and partition windows must
                # be 32-aligned), so write the g=0 pattern everywhere and add
                # the g=1 half-offset (H2*S = 208 for both split heads) from
                # consts col 7 as a one-time per-partition scalar.
                nc.gpsimd.iota(
                    gy[:, :], pattern=[[S, H2], [0, H]], base=0,
                    channel_multiplier=0, allow_small_or_imprecise_dtypes=True,
                )
                if G == 2:
                    nc.vector.tensor_scalar(
                        out=gy[:, :], in0=gy[:, :],
                        scalar1=consts_t[0:P, 7:8], scalar2=None, op0=ALU.add,
                    )
                grids[name] = (gx, gy)

            rep_ctx = tc.For_i(0, reps, 1) if reps > 1 else contextlib.nullcontext()
            with rep_ctx:
                _emit_body(nc, tc, wpool, spool, consts_t, grids, ins, obs, oms, variant)
    nc.compile()
    return nc


def _emit_body(nc, tc, wpool, spool, consts_t, grids, ins, obs, oms, variant="full"):
            for hi, (name, H, S, G, FC) in enumerate(HEADS):
                HW = H * H
                P = 48 * G
                F = HW // G
                gx, gy = grids[name]
                in_r = ins[name][:].rearrange("b (a r) (g f) -> g b a r f", a=3, g=G)
                ob_r = obs[name][:].rearrange("r b a (g f) -> r g b a f", g=G)
                om_r = oms[name][:].rearrange("b a (g f) -> g b a f", g=G)
                cw, ch = 2 * hi, 2 * hi + 1

                for cs in range(0, F, FC):
                    ce = cs + FC
                    t = []
                    for k in range(15):
                        tk = wpool.tile([P, FC], F32, tag=f"r{k}")
                        if variant != "compute":
                            eng = nc.sync if k % 2 == 0 else nc.scalar
                            eng.dma_start(out=tk[:, :], in_=in_r[:, :, :, k, cs:ce])
                        t.append(tk)

                    if variant == "dma":
                        msk = wpool.tile([P, FC], U8, tag="msk")
                        nc.gpsimd.memset(msk[:, :], 0)
                        for ri in range(6):
                            eng = nc.sync if ri % 2 == 0 else nc.scalar
                            eng.dma_start(out=ob_r[ri, :, :, :, cs:ce], in_=t[ri][:, :])
                        nc.sync.dma_start(out=om_r[:, :, :, cs:ce], in_=msk[:, :])
                        continue

                    hw2 = wpool.tile([P, FC], F32, tag="hw2")
                    hh2 = wpool.tile([P, FC], F32, tag="hh2")
                    # ACT runs only Exp (single function table, no reloads);
                    # bias folds the anchor/2 multiplier: a/2*e^x = e^(x+ln(a/2))
                    nc.scalar.activation(hw2[:, :], t[3][:, :], ACTF.Exp, bias=consts_t[0:P, cw:cw + 1])
                    nc.scalar.activation(hh2[:, :], t[4][:, :], ACTF.Exp, bias=consts_t[0:P, ch:ch + 1])

                    # pre = grid + S*d, fused: (d * S) + grid, in place on t1/t2
                    nc.vector.scalar_tensor_tensor(
                        t[1][:, :], t[1][:, :], float(S), gx[:, cs:ce], ALU.mult, ALU.add)
                    nc.vector.scalar_tensor_tensor(
                        t[2][:, :], t[2][:, :], float(S), gy[:, cs:ce], ALU.mult, ALU.add)
                    x1 = wpool.tile([P, FC], F32, tag="x1")
                    y1 = wpool.tile([P, FC], F32, tag="y1")
                    nc.vector.tensor_tensor(x1[:, :], t[1][:, :], hw2[:, :], ALU.subtract)
                    nc.vector.tensor_tensor(y1[:, :], t[2][:, :], hh2[:, :], ALU.subtract)
                    # x2/y2 in place on t1/t2
                    nc.vector.tensor_tensor(t[1][:, :], t[1][:, :], hw2[:, :], ALU.add)
                    nc.vector.tensor_tensor(t[2][:, :], t[2][:, :], hh2[:, :], ALU.add)

                    # mask = iou > thresh (gpsimd, off the DVE critical path)
                    msk = wpool.tile([P, FC], U8, tag="msk")
                    nc.gpsimd.tensor_scalar(
                        out=msk[:, :], in0=t[0][:, :],
                        scalar1=consts_t[0:P, 6:7], scalar2=None, op0=ALU.is_gt)

                    # tournament argmax over class planes t[5..14]
                    c = t[5:]
                    m = []   # running max planes (in place on even class planes)
                    ix = []  # running argmax planes
                    for i in range(5):
                        gt = spool.tile([P, FC], U8, tag=f"gt{i}")
                        idx = spool.tile([P, FC], F32, tag=f"idx{i}")
                        nc.vector.tensor_tensor(gt[:, :], c[2 * i + 1][:, :], c[2 * i][:, :], ALU.is_gt)
                        nc.vector.tensor_tensor(c[2 * i][:, :], c[2 * i][:, :], c[2 * i + 1][:, :], ALU.max)
                        nc.vector.tensor_scalar(
                            out=idx[:, :], in0=gt[:, :],
                            scalar1=float(2 * i), scalar2=None, op0=ALU.add,
                        )
                        m.append(c[2 * i])
                        ix.append(idx)

                    def merge(i, j):
                        g2 = spool.tile([P, FC], U8, tag="gm")
                        nc.vector.tensor_tensor(g2[:, :], m[j][:, :], m[i][:, :], ALU.is_gt)
                        nc.vector.copy_predicated(ix[i][:, :], g2[:, :], ix[j][:, :])
                        nc.vector.tensor_tensor(m[i][:, :], m[i][:, :], m[j][:, :], ALU.max)

                    merge(0, 1)
                    merge(2, 3)
                    merge(0, 2)
                    merge(0, 4)

                    if variant != "compute":
                        for ri, plane in ((0, t[0]), (1, x1), (2, y1), (3, t[1]), (4, t[2]), (5, ix[0])):
                            nc.sync.dma_start(out=ob_r[ri, :, :, :, cs:ce], in_=plane[:, :])
                        nc.sync.dma_start(out=om_r[:, :, :, cs:ce], in_=msk[:, :])


_NC_CACHE = {}


def _get_nc(reps=1, variant="full"):
    key = (reps, variant)
    if key not in _NC_CACHE:
        _NC_CACHE[key] = _build_nc(reps, variant)
    return _NC_CACHE[key]


def _host_inputs(output13, output26, output52, anchors13, anchors26, anchors52, thresh):
    consts = np.zeros((96, 8), np.float32)
    amod = np.arange(96) % 3
    for hi, anc in enumerate((anchors13, anchors26, anchors52)):
        anc = np.asarray(anc, np.float32)
        consts[:, 2 * hi] = np.log(anc[:, 0] / 2.0)[amod]
        consts[:, 2 * hi + 1] = np.log(anc[:, 1] / 2.0)[amod]
    consts[:, 6] = np.float32(thresh)
    consts[:, 7] = np.where(np.arange(96) >= 48, 208.0, 0.0)
    full = {"13": np.asarray(output13, np.float32),
            "26": np.asarray(output26, np.float32),
            "52": np.asarray(output52, np.float32)}
    in_maps = []
    for cid in range(NCORES):
        m = {"consts": consts}
        for name in full:
            sl = np.ascontiguousarray(full[name][cid * BL:(cid + 1) * BL])
            m[f"in{name}"] = sl.reshape(BL, 45, -1)
        in_maps.append(m)
    return in_maps


def _assemble(results):
    boxes_parts = []
    mask_parts = []
    for name, H, S, G, FC in HEADS:
        ob = np.concatenate([r[f"ob{name}"] for r in results], axis=1)  # [6,128,3,HW]
        om = np.concatenate([r[f"om{name}"] for r in results], axis=0)  # [128,3,HW]
        boxes_parts.append(np.ascontiguousarray(ob.transpose(1, 3, 2, 0)).reshape(-1, 6))
        mask_parts.append((om.transpose(0, 2, 1) != 0).reshape(-1))
    return np.concatenate(boxes_parts, 0), np.concatenate(mask_parts, 0)


def _run(trace=False, **inputs):
    nc = _get_nc()
    in_maps = _host_inputs(**inputs)
    res = run_bass_kernel_spmd(nc, in_maps, list(range(NCORES)), trace=trace)
    out = _assemble(res.results)
    return out, res


def kernel(**inputs):
    out, _ = _run(trace=False, **inputs)
    return out


def kernel_traced(**inputs):
    return _run(trace=True, **inputs)


# revision 31
# speedup vs baseline: 2.9320x; 2.8395x over previous
"""YOLO-style detector decode kernel for Trainium2 (8 NeuronCores, SPMD).

Sharding: pure data parallel over the batch dim (128 -> 16 per core).

Layout: for each head (13/26/52) the host prepacks the 45 channels
(3 anchors x 15 roles) into [G, B, 3, nchunk, 15, FC] so that each SBUF
partition row (g, b, a) loads its whole chunk (15 roles x FC cells) as ONE
contiguous ~40KB DMA descriptor -- real TRN2 DMA is descriptor-rate-bound
(~38ns/descriptor), so small per-role descriptors are 6x slower than the
HBM roofline.  Outputs are likewise packed as [G, B, 3, nchunk, 7, FC]
(iou, x1, y1, x2, y2, kind, mask) and unpacked on the host with cheap numpy
transposes.

Decode per cell-anchor (partition p = g*48 + b*3 + a, free = cells):

  pre   = grid*stride + stride*d        (gpsimd iota grid + one fused DVE STT)
  half  = exp(dw + ln(anchor/2))        (single ACT op; bias folds the mul,
                                         ACT only ever runs Exp -> no
                                         activation-table reloads)
  x1/x2 = pre -/+ half                  (DVE)
  mask  = iou > thresh                  (gpsimd, off the DVE critical path)
  kind  = tournament argmax over the 10 class planes (DVE)
"""

import numpy as np

import concourse.bass as bass
import concourse.bacc as bacc
import concourse.mybir as mybir
from concourse.tile import TileContext
from concourse.bass_utils import run_bass_kernel_spmd

F32 = mybir.dt.float32
U8 = mybir.dt.uint8
ALU = mybir.AluOpType
ACTF = mybir.ActivationFunctionType

NCORES = 8
B = 128
BL = B // NCORES  # 16

# name, H(=W), stride, G (cell-space split to widen partitions), n free chunks
HEADS = [
    ("13", 13, 32, 1, 1),
    ("26", 26, 16, 2, 1),
    ("52", 52, 8, 2, 2),
]


def _build_nc(reps=1, variant="full"):
    # Bacc (not raw Bass): its compile() legalizes multi-wait sync_info into
    # event semaphores (hardware allows one wait per instruction).
    # reps>1 wraps the body in a For_i loop -- used only for benchmarking
    # (fixed host/proxy overhead cancels between reps=1 and reps=R runs).
    # variant: "full" | "dma" (no compute) | "compute" (no chunk DMAs).
    import contextlib
    nc = bacc.Bacc("TRN2", target_bir_lowering=False)
    ins = {}
    outs = {}
    for name, H, S, G, NCH in HEADS:
        HW = H * H
        FC = HW // (G * NCH)
        ins[name] = nc.declare_dram_parameter(
            f"in{name}", [G, BL, 3, NCH, 15, FC], F32, isOutput=False)
        outs[name] = nc.declare_dram_parameter(
            f"out{name}", [G, BL, 3, NCH, 7, FC], F32, isOutput=True)
    consts = nc.declare_dram_parameter("consts", [96, 8], F32, isOutput=False)

    with TileContext(nc) as tc:
        with (
            tc.tile_pool(name="persist", bufs=1) as ppool,
            tc.tile_pool(name="work", bufs=2) as wpool,
            tc.tile_pool(name="scratch", bufs=1) as spool,
        ):
            consts_t = ppool.tile([96, 8], F32, tag="consts")
            nc.sync.dma_start(out=consts_t[:, :], in_=consts[:])

            # Per-head grid tiles (stride-scaled cell coordinates), built once
            # on gpsimd via iota.  Values <= 408, exact in f32.
            grids = {}
            for name, H, S, G, NCH in HEADS:
                P = 48 * G
                H2 = H // G
                F = H2 * H
                gx = ppool.tile([P, F], F32, tag=f"gx{name}")
                gy = ppool.tile([P, F], F32, tag=f"gy{name}")
                nc.gpsimd.iota(
                    gx[:, :], pattern=[[0, H2], [S, H]], base=0,
                    channel_multiplier=0, allow_small_or_imprecise_dtypes=True,
                )
                # iota can't write at a partition offset (and partition windows
                # must be 32-aligned), so write the g=0 pattern everywhere and
                # add the g=1 half-offset (H2*S = 208 for both split heads)
                # from consts col 7 as a one-time per-partition scalar.
                nc.gpsimd.iota(
                    gy[:, :], pattern=[[S, H2], [0, H]], base=0,
                    channel_multiplier=0, allow_small_or_imprecise_dtypes=True,
                )
                if G == 2:
                    nc.vector.tensor_scalar(
                        out=gy[:, :], in0=gy[:, :],
                        scalar1=consts_t[0:P, 7:8], scalar2=None, op0=ALU.add,
                    )
                grids[name] = (gx, gy)

            rep_ctx = tc.For_i(0, reps, 1) if reps > 1 else contextlib.nullcontext()
            with rep_ctx:
                _emit_body(nc, tc, wpool, spool, consts_t, grids, ins, outs, variant)
    nc.compile()
    return nc


def _emit_body(nc, tc, wpool, spool, consts_t, grids, ins, outs, variant="full"):
    for hi, (name, H, S, G, NCH) in enumerate(HEADS):
        HW = H * H
        P = 48 * G
        F = HW // G
        FC = F // NCH
        gx, gy = grids[name]
        in_t = ins[name]
        out_t = outs[name]
        cw, ch = 2 * hi, 2 * hi + 1

        for c in range(NCH):
            cs = c * FC
            ce = cs + FC
            it = wpool.tile([P, 15 * FC], F32, tag="in")
            ot = wpool.tile([P, 7 * FC], F32, tag="out")
            if variant != "compute":
                # one ~(15*FC*4)B descriptor per partition row
                nc.sync.dma_start(out=it[:, :], in_=in_t[:, :, :, c, :, :])

            def sl(tile, r):
                return tile[:, r * FC:(r + 1) * FC]

            if variant == "dma":
                nc.gpsimd.tensor_copy(ot[:, 0:7 * FC], it[:, 0:7 * FC])
                nc.scalar.dma_start(out=out_t[:, :, :, c, :, :], in_=ot[:, :])
                continue

            hw2 = wpool.tile([P, FC], F32, tag="hw2")
            hh2 = wpool.tile([P, FC], F32, tag="hh2")
            # ACT runs only Exp (single function table, no reloads);
            # bias folds the anchor/2 multiplier: a/2*e^x = e^(x+ln(a/2))
            nc.scalar.activation(hw2[:, :], sl(it, 3), ACTF.Exp, bias=consts_t[0:P, cw:cw + 1])
            nc.scalar.activation(hh2[:, :], sl(it, 4), ACTF.Exp, bias=consts_t[0:P, ch:ch + 1])

            # pre = grid + S*d, fused STT, in place on the dx/dy role slices
            nc.vector.scalar_tensor_tensor(
                sl(it, 1), sl(it, 1), float(S), gx[:, cs:ce], ALU.mult, ALU.add)
            nc.vector.scalar_tensor_tensor(
                sl(it, 2), sl(it, 2), float(S), gy[:, cs:ce], ALU.mult, ALU.add)
            nc.vector.tensor_tensor(sl(ot, 1), sl(it, 1), hw2[:, :], ALU.subtract)
            nc.vector.tensor_tensor(sl(ot, 3), sl(it, 1), hw2[:, :], ALU.add)
            nc.vector.tensor_tensor(sl(ot, 2), sl(it, 2), hh2[:, :], ALU.subtract)
            nc.vector.tensor_tensor(sl(ot, 4), sl(it, 2), hh2[:, :], ALU.add)

            # iou passthrough + mask (gpsimd, off the DVE critical path)
            nc.gpsimd.tensor_copy(sl(ot, 0), sl(it, 0))
            nc.gpsimd.tensor_scalar(
                out=sl(ot, 6), in0=sl(it, 0),
                scalar1=consts_t[0:P, 6:7], scalar2=None, op0=ALU.is_gt)

            # tournament argmax over class role slices 5..14
            cls = [sl(it, 5 + j) for j in range(10)]
            m = []   # running max (in place on even class slices)
            ix = []  # running argmax
            for i in range(5):
                gt = spool.tile([P, FC], U8, tag=f"gt{i}")
                idx = spool.tile([P, FC], F32, tag=f"idx{i}")
                nc.vector.tensor_tensor(gt[:, :], cls[2 * i + 1], cls[2 * i], ALU.is_gt)
                nc.vector.tensor_tensor(cls[2 * i], cls[2 * i], cls[2 * i + 1], ALU.max)
                nc.vector.tensor_scalar(
                    out=idx[:, :], in0=gt[:, :],
                    scalar1=float(2 * i), scalar2=None, op0=ALU.add)
                m.append(cls[2 * i])
                ix.append(idx[:, :])

            def merge(i, j):
                g2 = spool.tile([P, FC], U8, tag="gm")
                nc.vector.tensor_tensor(g2[:, :], m[j], m[i], ALU.is_gt)
                nc.vector.copy_predicated(ix[i], g2[:, :], ix[j])
                nc.vector.tensor_tensor(m[i], m[i], m[j], ALU.max)

            merge(0, 1)
            merge(2, 3)
            merge(0, 2)
            # final merge writes kind straight into the out tile
            gf = spool.tile([P, FC], U8, tag="gm")
            nc.vector.tensor_tensor(gf[:, :], m[4], m[0], ALU.is_gt)
            nc.vector.tensor_copy(sl(ot, 5), ix[0])
            nc.vector.copy_predicated(sl(ot, 5), gf[:, :], ix[4])

            if variant != "compute":
                nc.scalar.dma_start(out=out_t[:, :, :, c, :, :], in_=ot[:, :])


_NC_CACHE = {}


def _get_nc(reps=1, variant="full"):
    key = (reps, variant)
    if key not in _NC_CACHE:
        _NC_CACHE[key] = _build_nc(reps, variant)
    return _NC_CACHE[key]


def _host_inputs(output13, output26, output52, anchors13, anchors26, anchors52, thresh):
    consts = np.zeros((96, 8), np.float32)
    amod = np.arange(96) % 3
    for hi, anc in enumerate((anchors13, anchors26, anchors52)):
        anc = np.asarray(anc, np.float32)
        consts[:, 2 * hi] = np.log(anc[:, 0] / 2.0)[amod]
        consts[:, 2 * hi + 1] = np.log(anc[:, 1] / 2.0)[amod]
    consts[:, 6] = np.float32(thresh)
    consts[:, 7] = np.where(np.arange(96) >= 48, 208.0, 0.0)
    full = {"13": np.asarray(output13, np.float32),
            "26": np.asarray(output26, np.float32),
            "52": np.asarray(output52, np.float32)}
    # prepack: [B,45,H,W] -> (g, b, a, c, k, f) so each partition row's chunk
    # (15 roles x FC cells) is one contiguous DMA descriptor
    packed = {}
    for name, H, S, G, NCH in HEADS:
        HW = H * H
        FC = HW // (G * NCH)
        x = full[name].reshape(B, 3, 15, G, NCH, FC)  # (b, a, k, g, c, f)
        packed[name] = x.transpose(3, 0, 1, 4, 2, 5)  # (g, b, a, c, k, f)
    in_maps = []
    for cid in range(NCORES):
        mcore = {"consts": consts}
        for name, H, S, G, NCH in HEADS:
            sl = packed[name][:, cid * BL:(cid + 1) * BL]
            mcore[f"in{name}"] = np.ascontiguousarray(sl)
        in_maps.append(mcore)
    return in_maps


def _assemble(results):
    boxes_parts = []
    mask_parts = []
    for name, H, S, G, NCH in HEADS:
        HW = H * H
        FC = HW // (G * NCH)
        # [G, BL, 3, NCH, 7, FC] per core, concat batch
        ob = np.concatenate([r[f"out{name}"] for r in results], axis=1)
        # (g, b, a, c, r, f) -> (b, g, c, f, a, r) so (g,c,f) flattens to hw
        arr = ob.transpose(1, 0, 3, 5, 2, 4).reshape(B, HW, 3, 7)
        boxes_parts.append(arr[..., :6].reshape(-1, 6))
        mask_parts.append(arr[..., 6].reshape(-1) != 0)
    boxes = np.ascontiguousarray(np.concatenate(boxes_parts, 0))
    mask = np.concatenate(mask_parts, 0)
    return boxes, mask


def _run(trace=False, **inputs):
    nc = _get_nc()
    in_maps = _host_inputs(**inputs)
    res = run_bass_kernel_spmd(nc, in_maps, list(range(NCORES)), trace=trace)
    out = _assemble(res.results)
    return out, res


def kernel(**inputs):
    out, _ = _run(trace=False, **inputs)
    return out


def kernel_traced(**inputs):
    return _run(trace=True, **inputs)
